# revision 1
# baseline (speedup 1.0000x reference)
"""Trainium2 Bass kernel v2 for nn_Network_28054726377822 (LSTM, B=64 T=1024 D=512 U=512 OUT=4).

Strategy (data-parallel over batch, 8 samples/core):
  - Inputs shipped raw: tx as fp32 [BL*T, D] natural layout (zero host prep);
    weights column-permuted + bf16 on host (tiny).
  - Phase 1 (interleaved with recurrence as PE filler): per (sample, 128-row
    t-block): DMA x -> PE-transpose to lhsT bf16 -> 16+4 MMs -> xz bf16 to
    DRAM scratch, layout [t, hb, b, f=gp*128+jl].
  - Phase 2: 1024-step recurrence. z PSUM tile [128, 512] (ONE bank):
    partition = 32*hb + b (4 col-tiling strips x 8 samples), free = gp*128+jl
    with gate order gp = [z1(x2-folded), z3, z2, z4].
      * xz_t injected via one identity matmul (start=True), then 16 RMMs
        (4 strips x 4 K-tiles, N=512 bf16) streaming R from SBUF.
      * ONE Sigmoid over all 4 gates (x2 fold on z1 makes sigma(2x) encode
        tanh); v1 = 2*S1-1 via one fused tensor_scalar.
      * c update on DVE; tanh(c) on ACT; h assembled directly in transposed
        layout: hT = S4^T (*) tanh(c)^T via two PE transposes + one DVE mul,
        so the next step's lhsT is a plain SBUF slice.
  - Phase 3 (host): out = softmax(h_last @ fc_w + fc_b) in fp32 numpy.

Self-contained: hardcodes all shapes; sharding/gather done here in numpy.
"""

import numpy as np
import ml_dtypes

B, T, D, U, OUT = 64, 1024, 512, 512, 4
TC = 64                   # xz staging chunk (timesteps per chunk load)
NCORES = 8
BL = B // NCORES          # 8 samples per core
HB = 4                    # hidden blocks of 128 (col-tiling strips)
G4 = 4 * U                # 2048
BF16 = ml_dtypes.bfloat16
TANH_DIRECT = True        # Tanh ACT for tanh(c); else 2*sigmoid(2c)-1
ABLATION = ""             # debug ablations disabled in shipped kernel
S_FP32 = True             # sigma(z) kept fp32 (avoids 2*sigma-1 bf16 cancellation)
NG = 1                    # batch groups per core (1 = single chain, lowest latency)
SPLITZ = True             # z split into 2 banks by gate halves

# gate order in packed layout: slot gp -> reference gate g
# gp0 = z1 (tanh gate, x2-folded), gp1 = z3 (forget), gp2 = z2, gp3 = z4
GORDER = (0, 2, 1, 3)


def _perm_cols():
    """new col hb*512 + gp*128 + jl  <-  old col g*512 + hb*128 + jl."""
    idx = np.empty(G4, dtype=np.int64)
    for hb in range(HB):
        for gp, g in enumerate(GORDER):
            for jl in range(128):
                idx[hb * 512 + gp * 128 + jl] = g * 512 + hb * 128 + jl
    return idx


_PERM = _perm_cols()
# x2 fold on the z1 slot (new col % 512 < 128) so sigmoid(2*z1) encodes tanh(z1)
_COLSCALE = np.where((np.arange(G4) % 512) < 128, 2.0, 1.0).astype(np.float32)


def _build_bass():
    import concourse.mybir as mybir
    import concourse.tile as tile
    from concourse import bacc
    from concourse.masks import make_identity
    from concourse.tile_rust import add_dep_helper

    dt = mybir.dt
    AFT = mybir.ActivationFunctionType
    MULT = mybir.AluOpType.mult
    SUB = mybir.AluOpType.subtract
    nc = bacc.Bacc("TRN2", target_bir_lowering=False, num_devices=NCORES)

    # ---- I/O ----
    tx_d = nc.dram_tensor("tx", [BL * T, D], dt.float32, kind="ExternalInput").ap()
    kern_d = nc.dram_tensor("kern_perm", [D, G4], dt.bfloat16, kind="ExternalInput").ap()
    r_d = nc.dram_tensor("r_perm", [D, G4], dt.bfloat16, kind="ExternalInput").ap()
    bias_d = nc.dram_tensor("bias_perm", [1, G4], dt.bfloat16, kind="ExternalInput").ap()
    hT_out_d = nc.dram_tensor("hT_out", [NG, 128, 128], dt.float32, kind="ExternalOutput").ap()
    # DRAM scratch for xz, layout [t, hb, b, f(gp*128+jl)]
    xz_d = nc.dram_tensor("xz_scratch", [T, HB, BL, 512], dt.bfloat16, kind="Internal").ap()

    with tile.TileContext(nc) as tc:
        const = tc.tile_pool(name="const", bufs=1)
        with const as cpool:
            kern_sb = cpool.tile([128, 4, G4], dt.bfloat16, tag="kern")
            r_sb = cpool.tile([128, 4, G4], dt.bfloat16, tag="rsb")
            for k in range(4):
                nc.gpsimd.dma_start(out=kern_sb[:, k, :], in_=kern_d[128 * k : 128 * k + 128, :])
                nc.gpsimd.dma_start(out=r_sb[:, k, :], in_=r_d[128 * k : 128 * k + 128, :])
            bias_sb = cpool.tile([1, G4], dt.bfloat16, tag="bias")
            nc.gpsimd.dma_start(out=bias_sb, in_=bias_d)
            ones_sb = cpool.tile([1, 128], dt.bfloat16, tag="ones")
            nc.vector.memset(ones_sb, 1.0)
            ident_bf = cpool.tile([128, 128], dt.bfloat16, tag="identb")
            make_identity(nc, ident_bf)
            ident_f = cpool.tile([128, 128], dt.float32, tag="identf")
            make_identity(nc, ident_f)

            # persistent recurrence state (manually double-buffered), per group
            hT_g = []
            c_g = []
            for g in range(NG):
                hT_sb = cpool.tile([128, 2, 128], dt.bfloat16, tag=f"hT{g}")
                nc.vector.memset(hT_sb, 0.0)
                hT_g.append(hT_sb)
                c_sb = cpool.tile([128, 2, 128], dt.float32, tag=f"c{g}")
                nc.vector.memset(c_sb, 0.0)
                c_g.append(c_sb)
            # xz staging: double-buffered TC-step chunks; memset once so the
            # unused partitions (24 of every 32) hold zeros forever.
            xz_chunk = cpool.tile([128, 2, TC, 512], dt.bfloat16, tag="xzs")
            for sl in range(2):
                nc.vector.memset(xz_chunk[:, sl], 0.0)

            with tc.tile_pool(name="p1ps", bufs=1, space="PSUM") as p1ps, \
                 tc.tile_pool(name="p1zps", bufs=2, space="PSUM") as p1zps, \
                 tc.tile_pool(name="p1sb", bufs=3) as p1sb, \
                 tc.tile_pool(name="p2ps", bufs=1 if (SPLITZ or NG > 1) else 2, space="PSUM") as p2ps, \
                 tc.tile_pool(name="p2t", bufs=3, space="PSUM") as p2t, \
                 tc.tile_pool(name="p2sb", bufs=3) as p2sb:

                out_dmas_by_tb = {}

                def p1_slices():
                    """Each yield issues one hb-chunk of one (sample, t-block):
                    5 MMs + copy + out-DMA; first chunk also DMA-in + 4
                    transposes building the bf16 lhsT."""
                    state = {}
                    for tb in range(T // 128):
                        for b_i in range(BL):
                            for j in range(HB):
                                if j == 0:
                                    x_sb = p1sb.tile([128, 512], dt.float32, tag="xin")
                                    nc.sync.dma_start(
                                        out=x_sb,
                                        in_=tx_d[b_i * T + tb * 128 : b_i * T + tb * 128 + 128, :],
                                    )
                                    lhs = p1sb.tile([128, 4, 128], dt.bfloat16, tag="lhs")
                                    for k in range(4):
                                        xT = p1ps.tile([128, 128], dt.float32, tag="xT")
                                        nc.tensor.transpose(
                                            xT, x_sb[:, 128 * k : 128 * k + 128], ident_f
                                        )
                                        nc.vector.tensor_copy(lhs[:, k, :], xT)
                                    state["lhs"] = lhs
                                lhs = state["lhs"]
                                ps = p1zps.tile([128, 512], dt.float32, tag="p1z")
                                for k in range(4):
                                    nc.tensor.matmul(
                                        ps,
                                        lhsT=lhs[:, k, :],
                                        rhs=kern_sb[:, k, j * 512 : j * 512 + 512],
                                        start=(k == 0),
                                        stop=False,
                                        skip_group_check=True,
                                    )
                                nc.tensor.matmul(
                                    ps,
                                    lhsT=ones_sb,
                                    rhs=bias_sb[:, j * 512 : j * 512 + 512],
                                    start=False,
                                    stop=True,
                                    skip_group_check=True,
                                )
                                xzo = p1sb.tile([128, 512], dt.bfloat16, tag="xzo")
                                nc.vector.tensor_copy(xzo, ps)
                                od = nc.sync.dma_start(
                                    out=xz_d[tb * 128 : tb * 128 + 128, j, b_i, :], in_=xzo
                                )
                                out_dmas_by_tb.setdefault(tb, []).append(od)
                                yield

                p1_iter = p1_slices()

                def drip(n):
                    for _ in range(n):
                        if next(p1_iter, "done") == "done":
                            return

                # prime: everything for tb 0,1 (covers xz chunks 0..3)
                drip(2 * BL * HB)

                def load_chunk(ci):
                    tb_src = (ci * TC) // 128
                    for hb in range(HB):
                        cd = nc.sync.dma_start(
                            out=xz_chunk[32 * hb : 32 * hb + BL, ci % 2],
                            in_=xz_d[ci * TC : (ci + 1) * TC, hb].rearrange(
                                "t b f -> b t f"
                            ),
                        )
                        for od in out_dmas_by_tb.get(tb_src, []):
                            add_dep_helper(cd.ins, od.ins, sync=True,
                                           reason="xz RAW p1->p2")

                load_chunk(0)
                for t in range(T):
                    cur, nxt = t % 2, (t + 1) % 2
                    tc_i, tl = t // TC, t % TC
                    slot = tc_i % 2
                    if tl == 0 and tc_i + 1 < T // TC:
                        load_chunk(tc_i + 1)   # prefetch one chunk ahead
                    xz_sb = xz_chunk[:, slot, tl, :]

                    # phase-major emission: the two groups' chains interleave
                    # inside each engine's strict FIFO instead of serializing.
                    zs, Ss, S4Ts, S4T_sbs, tcts = {}, {}, {}, {}, {}
                    for g in range(NG):
                        hT_sb = hT_g[g]
                        # -- z = xz + h @ R: inject + 16 RMMs into ONE psum bank
                        # (rows outside this group's samples carry bounded junk)
                        if SPLITZ:
                            halves = []
                            for hv in range(2):
                                zh = p2ps.tile([128, 256], dt.float32,
                                               tag=f"z{g}h{hv}", name=f"z{g}h{hv}")
                                halves.append(zh)
                                nc.tensor.matmul(
                                    zh, lhsT=ident_bf,
                                    rhs=xz_sb[:, 256 * hv : 256 * hv + 256],
                                    start=True, stop=False, skip_group_check=True,
                                )
                                for k in range(4):
                                    for hb in range(HB):
                                        nc.tensor.matmul(
                                            zh[32 * hb : 32 * hb + BL, :],
                                            lhsT=hT_sb[:, cur, 32 * k : 32 * k + BL],
                                            rhs=r_sb[:, k, hb * 512 + 256 * hv : hb * 512 + 256 * hv + 256],
                                            start=False,
                                            stop=(hb == 3 and k == 3),
                                            skip_group_check=True,
                                            tile_position=(0, 32 * hb),
                                        )
                            zs[g] = halves
                        else:
                            z = p2ps.tile([128, 512], dt.float32, tag=f"z{g}", name=f"z{g}")
                            zs[g] = z
                            nc.tensor.matmul(
                                z, lhsT=ident_bf, rhs=xz_sb,
                                start=True, stop=False, skip_group_check=True,
                            )
                            # k-outer / strip-inner: consecutive MMs hit different
                            # col strips AND different PSUM rows, so weight loads
                            # overlap in-flight MMs and accumulate RMWs pipeline.
                            for k in range(4):
                                for hb in range(HB):
                                    nc.tensor.matmul(
                                        z[32 * hb : 32 * hb + BL, :],
                                        lhsT=hT_sb[:, cur, 32 * k : 32 * k + BL],
                                        rhs=r_sb[:, k, hb * 512 : hb * 512 + 512],
                                        start=False,
                                        stop=(hb == 3 and k == 3),
                                        skip_group_check=True,
                                        tile_position=(0, 32 * hb),
                                    )

                    # -- PE filler: drip phase-1 slices into the tail stall
                    # (emitted here so drip MMs precede next step's RMMs in
                    # the PE FIFO but run during this step's tail idle) --
                    if t % 4 == 0:
                        drip(1)
                    if ABLATION == "rmm_only":
                        continue
                    for g in range(NG):
                        # -- sigmoid over all gates (one instr, or per half) --
                        S = p2sb.tile([128, 512], dt.float32 if S_FP32 else dt.bfloat16, tag=f"S{g}", name=f"S{g}")
                        Ss[g] = S
                        if SPLITZ:
                            nc.scalar.activation(S[:, 0:256], zs[g][0], AFT.Sigmoid)
                            nc.scalar.activation(S[:, 256:512], zs[g][1], AFT.Sigmoid)
                        else:
                            nc.scalar.activation(S, zs[g], AFT.Sigmoid)
                    for g in range(NG):
                        # -- cell update (DVE) --
                        S, c_sb = Ss[g], c_g[g]
                        w1 = p2sb.tile([128, 128], dt.bfloat16, tag=f"w1{g}", name=f"w1{g}")
                        nc.vector.tensor_scalar(w1, S[:, 0:128], 2.0, 1.0, MULT, SUB)
                        m2 = p2sb.tile([128, 128], dt.float32, tag=f"m2{g}", name=f"m2{g}")
                        nc.vector.tensor_mul(m2, S[:, 128:256], c_sb[:, cur, :])
                        m1 = p2sb.tile([128, 128], dt.float32, tag=f"m1{g}", name=f"m1{g}")
                        nc.vector.tensor_mul(m1, w1, S[:, 256:384])
                        nc.vector.tensor_add(c_sb[:, nxt, :], m1, m2)

                    if ABLATION == "no_hT":
                        continue
                    for g in range(NG):
                        # -- S4^T via PE, copied to SBUF on ScalarE --
                        S = Ss[g]
                        S4T = p2t.tile([128, 128], dt.float32 if S_FP32 else dt.bfloat16, tag="tp", name=f"S4T{g}")
                        S4Ts[g] = S4T
                        nc.tensor.transpose(S4T, S[:, 384:512], ident_f if S_FP32 else ident_bf)
                    for g in range(NG):
                        S4T_sb = p2sb.tile([128, 128], dt.bfloat16, tag=f"s4t{g}", name=f"s4t{g}")
                        S4T_sbs[g] = S4T_sb
                        nc.scalar.copy(S4T_sb, S4Ts[g])
                    for g in range(NG):
                        tc_t = p2sb.tile([128, 128], dt.bfloat16, tag=f"tct{g}", name=f"tct{g}")
                        tcts[g] = tc_t
                        nc.scalar.activation(tc_t, c_g[g][:, nxt, :], AFT.Tanh)
                    for g in range(NG):
                        tcT = p2t.tile([128, 128], dt.bfloat16, tag="tp", name=f"tcT{g}")
                        nc.tensor.transpose(tcT, tcts[g], ident_bf)
                        nc.vector.tensor_mul(hT_g[g][:, nxt, :], S4T_sbs[g], tcT)

                drip(10**9)

            tc.strict_bb_all_engine_barrier()
            hT_f = cpool.tile([128, NG, 128], dt.float32, tag="hTf")
            for g in range(NG):
                nc.vector.tensor_copy(hT_f[:, g, :], hT_g[g][:, T % 2, :])
            for g in range(NG):
                nc.sync.dma_start(out=hT_out_d[g], in_=hT_f[:, g, :])

    nc.compile()
    return nc


_NC_CACHE = None
LAST_RESULTS = None  # BassKernelResults from the most recent run (for profiling)


def _make_in_maps(tx, kern, R, bias):
    kern_perm = np.ascontiguousarray(kern[:, _PERM] * _COLSCALE).astype(BF16)
    r_perm = np.ascontiguousarray(R[:, _PERM] * _COLSCALE).astype(BF16)
    bias_perm = np.ascontiguousarray(bias[_PERM] * _COLSCALE)[None, :].astype(BF16)
    in_maps = []
    for ci in range(NCORES):
        in_maps.append({
            "tx": np.ascontiguousarray(tx[ci * BL : (ci + 1) * BL].reshape(BL * T, D)),
            "kern_perm": kern_perm,
            "r_perm": r_perm,
            "bias_perm": bias_perm,
        })
    return in_maps


def _gather_h(per_core_results):
    h_last = np.empty((B, U), dtype=np.float32)
    GB = BL // NG
    for ci in range(NCORES):
        hT = per_core_results[ci]["hT_out"]          # [NG, jl, 32*hb+b] f32
        for b in range(BL):
            g = b // GB
            h_last[ci * BL + b] = hT[g].reshape(128, HB, 32)[:, :, b].T.reshape(U)
    return h_last


def kernel(tx, kernel, recurrent_kernel, bias, fc_w, fc_b):
    global _NC_CACHE, LAST_RESULTS
    from concourse.bass_utils import run_bass_kernel_spmd

    tx = np.asarray(tx, dtype=np.float32)
    kern = np.asarray(kernel, dtype=np.float32)
    R = np.asarray(recurrent_kernel, dtype=np.float32)
    bias = np.asarray(bias, dtype=np.float32)
    fc_w = np.asarray(fc_w, dtype=np.float32)
    fc_b = np.asarray(fc_b, dtype=np.float32)

    if _NC_CACHE is None:
        _NC_CACHE = _build_bass()
    nc = _NC_CACHE

    in_maps = _make_in_maps(tx, kern, R, bias)
    res = run_bass_kernel_spmd(nc, in_maps, core_ids=list(range(NCORES)))
    LAST_RESULTS = res
    h_last = _gather_h(res.results)

    logits = h_last @ fc_w + fc_b
    e = np.exp(logits - logits.max(axis=1, keepdims=True))
    return (e / e.sum(axis=1, keepdims=True)).astype(np.float32)



# revision 3
# speedup vs baseline: 1.0088x; 1.0088x over previous
"""Trainium2 Bass kernel v2 for nn_Network_28054726377822 (LSTM, B=64 T=1024 D=512 U=512 OUT=4).

Strategy (data-parallel over batch, 8 samples/core):
  - Inputs shipped raw: tx as fp32 [BL*T, D] natural layout (zero host prep);
    weights column-permuted + bf16 on host (tiny).
  - Phase 1 (interleaved with recurrence as PE filler): per (sample, 128-row
    t-block): DMA x -> PE-transpose to lhsT bf16 -> 16+4 MMs -> xz bf16 to
    DRAM scratch, layout [t, hb, b, f=gp*128+jl].
  - Phase 2: 1024-step recurrence. z PSUM tile [128, 512] (ONE bank):
    partition = 32*hb + b (4 col-tiling strips x 8 samples), free = gp*128+jl
    with gate order gp = [z1(x2-folded), z3, z2, z4].
      * xz_t injected via one identity matmul (start=True), then 16 RMMs
        (4 strips x 4 K-tiles, N=512 bf16) streaming R from SBUF.
      * ONE Sigmoid over all 4 gates (x2 fold on z1 makes sigma(2x) encode
        tanh); v1 = 2*S1-1 via one fused tensor_scalar.
      * c update on DVE; tanh(c) on ACT; h assembled directly in transposed
        layout: hT = S4^T (*) tanh(c)^T via two PE transposes + one DVE mul,
        so the next step's lhsT is a plain SBUF slice.
  - Phase 3 (host): out = softmax(h_last @ fc_w + fc_b) in fp32 numpy.

Self-contained: hardcodes all shapes; sharding/gather done here in numpy.
"""

import os
import numpy as np
import ml_dtypes

B, T, D, U, OUT = 64, 1024, 512, 512, 4
TC = 64                   # xz staging chunk (timesteps per chunk load)
NCORES = 8
BL = B // NCORES          # 8 samples per core
HB = 4                    # hidden blocks of 128 (col-tiling strips)
G4 = 4 * U                # 2048
BF16 = ml_dtypes.bfloat16
TANH_DIRECT = True        # Tanh ACT for tanh(c); else 2*sigmoid(2c)-1
ABLATION = os.environ.get("V2_ABLATION", "")  # debug ablations (empty in harness)
S_FP32 = True             # sigma(z) kept fp32 (avoids 2*sigma-1 bf16 cancellation)
NG = 1                    # batch groups per core (1 = single chain, lowest latency)
SPLITZ = True             # z split into 2 banks by gate halves

# gate order in packed layout: slot gp -> reference gate g
# gp0 = z1 (tanh gate, x2-folded), gp1 = z3 (forget), gp2 = z2, gp3 = z4
GORDER = (0, 2, 1, 3)


def _perm_cols():
    """new col hb*512 + gp*128 + jl  <-  old col g*512 + hb*128 + jl."""
    idx = np.empty(G4, dtype=np.int64)
    for hb in range(HB):
        for gp, g in enumerate(GORDER):
            for jl in range(128):
                idx[hb * 512 + gp * 128 + jl] = g * 512 + hb * 128 + jl
    return idx


_PERM = _perm_cols()
# x2 fold on the z1 slot (new col % 512 < 128) so sigmoid(2*z1) encodes tanh(z1)
_COLSCALE = np.where((np.arange(G4) % 512) < 128, 2.0, 1.0).astype(np.float32)


def _build_bass():
    import concourse.mybir as mybir
    import concourse.tile as tile
    from concourse import bacc
    from concourse.masks import make_identity
    from concourse.tile_rust import add_dep_helper

    dt = mybir.dt
    AFT = mybir.ActivationFunctionType
    MULT = mybir.AluOpType.mult
    SUB = mybir.AluOpType.subtract
    nc = bacc.Bacc("TRN2", target_bir_lowering=False, num_devices=NCORES)

    # ---- I/O ----
    tx_d = nc.dram_tensor("tx", [BL * T, D], dt.float32, kind="ExternalInput").ap()
    kern_d = nc.dram_tensor("kern_perm", [D, G4], dt.bfloat16, kind="ExternalInput").ap()
    r_d = nc.dram_tensor("r_perm", [D, G4], dt.bfloat16, kind="ExternalInput").ap()
    bias_d = nc.dram_tensor("bias_perm", [1, G4], dt.bfloat16, kind="ExternalInput").ap()
    hT_out_d = nc.dram_tensor("hT_out", [NG, 128, 128], dt.float32, kind="ExternalOutput").ap()
    # DRAM scratch for xz, layout [t, hb, b, f(gp*128+jl)]
    xz_d = nc.dram_tensor("xz_scratch", [T, HB, BL, 512], dt.bfloat16, kind="Internal").ap()

    with tile.TileContext(nc) as tc:
        const = tc.tile_pool(name="const", bufs=1)
        with const as cpool:
            kern_sb = cpool.tile([128, 4, G4], dt.bfloat16, tag="kern")
            r_sb = cpool.tile([128, 4, G4], dt.bfloat16, tag="rsb")
            for k in range(4):
                nc.gpsimd.dma_start(out=kern_sb[:, k, :], in_=kern_d[128 * k : 128 * k + 128, :])
                nc.gpsimd.dma_start(out=r_sb[:, k, :], in_=r_d[128 * k : 128 * k + 128, :])
            bias_sb = cpool.tile([1, G4], dt.bfloat16, tag="bias")
            nc.gpsimd.dma_start(out=bias_sb, in_=bias_d)
            ones_sb = cpool.tile([1, 128], dt.bfloat16, tag="ones")
            nc.vector.memset(ones_sb, 1.0)
            ident_bf = cpool.tile([128, 128], dt.bfloat16, tag="identb")
            make_identity(nc, ident_bf)
            ident_f = cpool.tile([128, 128], dt.float32, tag="identf")
            make_identity(nc, ident_f)

            # persistent recurrence state (manually double-buffered), per group
            hT_g = []
            c_g = []
            for g in range(NG):
                hT_sb = cpool.tile([128, 2, 128], dt.bfloat16, tag=f"hT{g}")
                nc.vector.memset(hT_sb, 0.0)
                hT_g.append(hT_sb)
                c_sb = cpool.tile([128, 2, 128], dt.float32, tag=f"c{g}")
                nc.vector.memset(c_sb, 0.0)
                c_g.append(c_sb)
            # xz staging: double-buffered TC-step chunks; memset once so the
            # unused partitions (24 of every 32) hold zeros forever.
            xz_chunk = cpool.tile([128, 2, TC, 512], dt.bfloat16, tag="xzs")
            for sl in range(2):
                nc.vector.memset(xz_chunk[:, sl], 0.0)

            with tc.tile_pool(name="p1ps", bufs=1, space="PSUM") as p1ps, \
                 tc.tile_pool(name="p1zps", bufs=2, space="PSUM") as p1zps, \
                 tc.tile_pool(name="p1sb", bufs=3) as p1sb, \
                 tc.tile_pool(name="p2ps", bufs=1 if (SPLITZ or NG > 1) else 2, space="PSUM") as p2ps, \
                 tc.tile_pool(name="p2t", bufs=3, space="PSUM") as p2t, \
                 tc.tile_pool(name="p2sb", bufs=3) as p2sb:

                out_dmas_by_tb = {}

                def p1_slices():
                    """Each yield issues one hb-chunk of one (sample, t-block):
                    5 MMs + copy + out-DMA; first chunk also DMA-in + 4
                    transposes building the bf16 lhsT."""
                    state = {}
                    for tb in range(T // 128):
                        for b_i in range(BL):
                            for j in range(HB):
                                if j == 0:
                                    x_sb = p1sb.tile([128, 512], dt.float32, tag="xin")
                                    nc.sync.dma_start(
                                        out=x_sb,
                                        in_=tx_d[b_i * T + tb * 128 : b_i * T + tb * 128 + 128, :],
                                    )
                                    lhs = p1sb.tile([128, 4, 128], dt.bfloat16, tag="lhs")
                                    for k in range(4):
                                        xT = p1ps.tile([128, 128], dt.float32, tag="xT")
                                        nc.tensor.transpose(
                                            xT, x_sb[:, 128 * k : 128 * k + 128], ident_f
                                        )
                                        nc.vector.tensor_copy(lhs[:, k, :], xT)
                                    state["lhs"] = lhs
                                lhs = state["lhs"]
                                ps = p1zps.tile([128, 512], dt.float32, tag="p1z")
                                for k in range(4):
                                    nc.tensor.matmul(
                                        ps,
                                        lhsT=lhs[:, k, :],
                                        rhs=kern_sb[:, k, j * 512 : j * 512 + 512],
                                        start=(k == 0),
                                        stop=False,
                                        skip_group_check=True,
                                    )
                                nc.tensor.matmul(
                                    ps,
                                    lhsT=ones_sb,
                                    rhs=bias_sb[:, j * 512 : j * 512 + 512],
                                    start=False,
                                    stop=True,
                                    skip_group_check=True,
                                )
                                xzo = p1sb.tile([128, 512], dt.bfloat16, tag="xzo")
                                nc.vector.tensor_copy(xzo, ps)
                                od = nc.sync.dma_start(
                                    out=xz_d[tb * 128 : tb * 128 + 128, j, b_i, :], in_=xzo
                                )
                                out_dmas_by_tb.setdefault(tb, []).append(od)
                                yield

                p1_iter = p1_slices()

                def drip(n):
                    for _ in range(n):
                        if next(p1_iter, "done") == "done":
                            return

                # prime: everything for tb 0,1 (covers xz chunks 0..3)
                drip(2 * BL * HB)

                def load_chunk(ci):
                    tb_src = (ci * TC) // 128
                    for hb in range(HB):
                        cd = nc.sync.dma_start(
                            out=xz_chunk[32 * hb : 32 * hb + BL, ci % 2],
                            in_=xz_d[ci * TC : (ci + 1) * TC, hb].rearrange(
                                "t b f -> b t f"
                            ),
                        )
                        for od in out_dmas_by_tb.get(tb_src, []):
                            add_dep_helper(cd.ins, od.ins, sync=True,
                                           reason="xz RAW p1->p2")

                load_chunk(0)
                for t in range(T):
                    cur, nxt = t % 2, (t + 1) % 2
                    tc_i, tl = t // TC, t % TC
                    slot = tc_i % 2
                    if tl == 0 and tc_i + 1 < T // TC:
                        load_chunk(tc_i + 1)   # prefetch one chunk ahead
                    xz_sb = xz_chunk[:, slot, tl, :]

                    # phase-major emission: the two groups' chains interleave
                    # inside each engine's strict FIFO instead of serializing.
                    zs, Ss, S4Ts, S4T_sbs, tcts = {}, {}, {}, {}, {}
                    for g in range(NG):
                        hT_sb = hT_g[g]
                        # -- z = xz + h @ R: inject + 16 RMMs into ONE psum bank
                        # (rows outside this group's samples carry bounded junk)
                        if SPLITZ:
                            # both injects first (one ident lhsT load, not
                            # two), half-B k-loop descending so it reuses
                            # half-A's last-loaded k3 slice: 8 lhsT changes
                            # per step instead of 10 (~146ns each measured).
                            halves = []
                            for hv in range(2):
                                zh = p2ps.tile([128, 256], dt.float32,
                                               tag=f"z{g}h{hv}", name=f"z{g}h{hv}")
                                halves.append(zh)
                                nc.tensor.matmul(
                                    zh, lhsT=ident_bf,
                                    rhs=xz_sb[:, 256 * hv : 256 * hv + 256],
                                    start=True, stop=False, skip_group_check=True,
                                )
                            for hv in range(2):
                                korder = range(4) if hv == 0 else range(3, -1, -1)
                                for k in korder:
                                    for hb in range(HB):
                                        nc.tensor.matmul(
                                            halves[hv][32 * hb : 32 * hb + BL, :],
                                            lhsT=hT_sb[:, cur, 32 * k : 32 * k + BL],
                                            rhs=r_sb[:, k, hb * 512 + 256 * hv : hb * 512 + 256 * hv + 256],
                                            start=False,
                                            stop=(hb == 3 and (k == 3 if hv == 0 else k == 0)),
                                            skip_group_check=True,
                                            tile_position=(0, 32 * hb),
                                        )
                            zs[g] = halves
                        else:
                            z = p2ps.tile([128, 512], dt.float32, tag=f"z{g}", name=f"z{g}")
                            zs[g] = z
                            nc.tensor.matmul(
                                z, lhsT=ident_bf, rhs=xz_sb,
                                start=True, stop=False, skip_group_check=True,
                            )
                            # k-outer / strip-inner: consecutive MMs hit different
                            # col strips AND different PSUM rows, so weight loads
                            # overlap in-flight MMs and accumulate RMWs pipeline.
                            for k in range(4):
                                for hb in range(HB):
                                    nc.tensor.matmul(
                                        z[32 * hb : 32 * hb + BL, :],
                                        lhsT=hT_sb[:, cur, 32 * k : 32 * k + BL],
                                        rhs=r_sb[:, k, hb * 512 : hb * 512 + 512],
                                        start=False,
                                        stop=(hb == 3 and k == 3),
                                        skip_group_check=True,
                                        tile_position=(0, 32 * hb),
                                    )

                    # -- PE filler: drip phase-1 slices into the tail stall
                    # (emitted here so drip MMs precede next step's RMMs in
                    # the PE FIFO but run during this step's tail idle) --
                    if t % 4 == 0:
                        drip(1)
                    if ABLATION == "rmm_only":
                        continue
                    for g in range(NG):
                        # -- sigmoid over all gates (one instr, or per half) --
                        S = p2sb.tile([128, 512], dt.float32 if S_FP32 else dt.bfloat16, tag=f"S{g}", name=f"S{g}")
                        Ss[g] = S
                        if SPLITZ:
                            nc.scalar.activation(S[:, 0:256], zs[g][0], AFT.Sigmoid)
                            nc.scalar.activation(S[:, 256:512], zs[g][1], AFT.Sigmoid)
                        else:
                            nc.scalar.activation(S, zs[g], AFT.Sigmoid)
                    for g in range(NG):
                        # -- cell update (DVE) --
                        S, c_sb = Ss[g], c_g[g]
                        w1 = p2sb.tile([128, 128], dt.bfloat16, tag=f"w1{g}", name=f"w1{g}")
                        nc.vector.tensor_scalar(w1, S[:, 0:128], 2.0, 1.0, MULT, SUB)
                        m2 = p2sb.tile([128, 128], dt.float32, tag=f"m2{g}", name=f"m2{g}")
                        nc.vector.tensor_mul(m2, S[:, 128:256], c_sb[:, cur, :])
                        m1 = p2sb.tile([128, 128], dt.float32, tag=f"m1{g}", name=f"m1{g}")
                        nc.vector.tensor_mul(m1, w1, S[:, 256:384])
                        nc.vector.tensor_add(c_sb[:, nxt, :], m1, m2)

                    if ABLATION == "no_hT":
                        continue
                    for g in range(NG):
                        # -- S4^T via PE, copied to SBUF on ScalarE --
                        S = Ss[g]
                        S4T = p2t.tile([128, 128], dt.float32 if S_FP32 else dt.bfloat16, tag="tp", name=f"S4T{g}")
                        S4Ts[g] = S4T
                        nc.tensor.transpose(S4T, S[:, 384:512], ident_f if S_FP32 else ident_bf)
                    for g in range(NG):
                        S4T_sb = p2sb.tile([128, 128], dt.bfloat16, tag=f"s4t{g}", name=f"s4t{g}")
                        S4T_sbs[g] = S4T_sb
                        nc.scalar.copy(S4T_sb, S4Ts[g])
                    for g in range(NG):
                        tc_t = p2sb.tile([128, 128], dt.bfloat16, tag=f"tct{g}", name=f"tct{g}")
                        tcts[g] = tc_t
                        nc.scalar.activation(tc_t, c_g[g][:, nxt, :], AFT.Tanh)
                    for g in range(NG):
                        tcT = p2t.tile([128, 128], dt.bfloat16, tag="tp", name=f"tcT{g}")
                        nc.tensor.transpose(tcT, tcts[g], ident_bf)
                        nc.vector.tensor_mul(hT_g[g][:, nxt, :], S4T_sbs[g], tcT)

                drip(10**9)

            tc.strict_bb_all_engine_barrier()
            hT_f = cpool.tile([128, NG, 128], dt.float32, tag="hTf")
            for g in range(NG):
                nc.vector.tensor_copy(hT_f[:, g, :], hT_g[g][:, T % 2, :])
            for g in range(NG):
                nc.sync.dma_start(out=hT_out_d[g], in_=hT_f[:, g, :])

    nc.compile()
    return nc


_NC_CACHE = None
LAST_RESULTS = None  # BassKernelResults from the most recent run (for profiling)


def _make_in_maps(tx, kern, R, bias):
    kern_perm = np.ascontiguousarray(kern[:, _PERM] * _COLSCALE).astype(BF16)
    r_perm = np.ascontiguousarray(R[:, _PERM] * _COLSCALE).astype(BF16)
    bias_perm = np.ascontiguousarray(bias[_PERM] * _COLSCALE)[None, :].astype(BF16)
    in_maps = []
    for ci in range(NCORES):
        in_maps.append({
            "tx": np.ascontiguousarray(tx[ci * BL : (ci + 1) * BL].reshape(BL * T, D)),
            "kern_perm": kern_perm,
            "r_perm": r_perm,
            "bias_perm": bias_perm,
        })
    return in_maps


def _gather_h(per_core_results):
    h_last = np.empty((B, U), dtype=np.float32)
    GB = BL // NG
    for ci in range(NCORES):
        hT = per_core_results[ci]["hT_out"]          # [NG, jl, 32*hb+b] f32
        for b in range(BL):
            g = b // GB
            h_last[ci * BL + b] = hT[g].reshape(128, HB, 32)[:, :, b].T.reshape(U)
    return h_last


def kernel(tx, kernel, recurrent_kernel, bias, fc_w, fc_b):
    global _NC_CACHE, LAST_RESULTS
    from concourse.bass_utils import run_bass_kernel_spmd

    tx = np.asarray(tx, dtype=np.float32)
    kern = np.asarray(kernel, dtype=np.float32)
    R = np.asarray(recurrent_kernel, dtype=np.float32)
    bias = np.asarray(bias, dtype=np.float32)
    fc_w = np.asarray(fc_w, dtype=np.float32)
    fc_b = np.asarray(fc_b, dtype=np.float32)

    if _NC_CACHE is None:
        _NC_CACHE = _build_bass()
    nc = _NC_CACHE

    in_maps = _make_in_maps(tx, kern, R, bias)
    res = run_bass_kernel_spmd(nc, in_maps, core_ids=list(range(NCORES)))
    LAST_RESULTS = res
    h_last = _gather_h(res.results)

    logits = h_last @ fc_w + fc_b
    e = np.exp(logits - logits.max(axis=1, keepdims=True))
    return (e / e.sum(axis=1, keepdims=True)).astype(np.float32)



# revision 4
# speedup vs baseline: 1.0340x; 1.0250x over previous
"""Trainium2 Bass kernel v2 for nn_Network_28054726377822 (LSTM, B=64 T=1024 D=512 U=512 OUT=4).

Strategy (data-parallel over batch, 8 samples/core):
  - Inputs shipped raw: tx as fp32 [BL*T, D] natural layout (zero host prep);
    weights column-permuted + bf16 on host (tiny).
  - Phase 1 (interleaved with recurrence as PE filler): per (sample, 128-row
    t-block): DMA x -> PE-transpose to lhsT bf16 -> 16+4 MMs -> xz bf16 to
    DRAM scratch, layout [t, hb, b, f=gp*128+jl].
  - Phase 2: 1024-step recurrence. z PSUM tile [128, 512] (ONE bank):
    partition = 32*hb + b (4 col-tiling strips x 8 samples), free = gp*128+jl
    with gate order gp = [z1(x2-folded), z3, z2, z4].
      * xz_t injected via one identity matmul (start=True), then 16 RMMs
        (4 strips x 4 K-tiles, N=512 bf16) streaming R from SBUF.
      * ONE Sigmoid over all 4 gates (x2 fold on z1 makes sigma(2x) encode
        tanh); v1 = 2*S1-1 via one fused tensor_scalar.
      * c update on DVE; tanh(c) on ACT; h assembled directly in transposed
        layout: hT = S4^T (*) tanh(c)^T via two PE transposes + one DVE mul,
        so the next step's lhsT is a plain SBUF slice.
  - Phase 3 (host): out = softmax(h_last @ fc_w + fc_b) in fp32 numpy.

Self-contained: hardcodes all shapes; sharding/gather done here in numpy.
"""

import os
import numpy as np
import ml_dtypes

B, T, D, U, OUT = 64, 1024, 512, 512, 4
TC = 64                   # xz staging chunk (timesteps per chunk load)
NCORES = 8
BL = B // NCORES          # 8 samples per core
HB = 4                    # hidden blocks of 128 (col-tiling strips)
G4 = 4 * U                # 2048
BF16 = ml_dtypes.bfloat16
TANH_DIRECT = True        # Tanh ACT for tanh(c); else 2*sigmoid(2c)-1
ABLATION = os.environ.get("V2_ABLATION", "")  # debug ablations (empty in harness)
S_FP32 = True             # sigma(z) kept fp32 (avoids 2*sigma-1 bf16 cancellation)
NG = 1                    # batch groups per core (1 = single chain, lowest latency)
SPLITZ = True             # z split into 2 banks by gate halves

# gate order in packed layout: slot gp -> reference gate g
# gp0 = z1 (tanh gate, x2-folded), gp1 = z3 (forget), gp2 = z2, gp3 = z4
GORDER = (0, 2, 1, 3)


def _perm_cols():
    """new col hb*512 + gp*128 + jl  <-  old col g*512 + hb*128 + jl."""
    idx = np.empty(G4, dtype=np.int64)
    for hb in range(HB):
        for gp, g in enumerate(GORDER):
            for jl in range(128):
                idx[hb * 512 + gp * 128 + jl] = g * 512 + hb * 128 + jl
    return idx


_PERM = _perm_cols()
# x2 fold on the z1 slot (new col % 512 < 128) so sigmoid(2*z1) encodes tanh(z1)
_COLSCALE = np.where((np.arange(G4) % 512) < 128, 2.0, 1.0).astype(np.float32)


def _build_bass():
    import concourse.mybir as mybir
    import concourse.tile as tile
    from concourse import bacc
    from concourse.masks import make_identity
    from concourse.tile_rust import add_dep_helper

    dt = mybir.dt
    AFT = mybir.ActivationFunctionType
    MULT = mybir.AluOpType.mult
    SUB = mybir.AluOpType.subtract
    nc = bacc.Bacc("TRN2", target_bir_lowering=False, num_devices=NCORES)

    # ---- I/O ----
    tx_d = nc.dram_tensor("tx", [BL * T, D], dt.float32, kind="ExternalInput").ap()
    kern_d = nc.dram_tensor("kern_perm", [D, G4], dt.bfloat16, kind="ExternalInput").ap()
    r_d = nc.dram_tensor("r_perm", [D, G4], dt.bfloat16, kind="ExternalInput").ap()
    bias_d = nc.dram_tensor("bias_perm", [1, G4], dt.bfloat16, kind="ExternalInput").ap()
    hT_out_d = nc.dram_tensor("hT_out", [NG, 128, 128], dt.float32, kind="ExternalOutput").ap()
    # DRAM scratch for xz, layout [t, hb, b, f(gp*128+jl)]
    xz_d = nc.dram_tensor("xz_scratch", [T, HB, BL, 512], dt.bfloat16, kind="Internal").ap()

    with tile.TileContext(nc) as tc:
        const = tc.tile_pool(name="const", bufs=1)
        with const as cpool:
            kern_sb = cpool.tile([128, 4, G4], dt.bfloat16, tag="kern")
            r_sb = cpool.tile([128, 4, G4], dt.bfloat16, tag="rsb")
            for k in range(4):
                nc.gpsimd.dma_start(out=kern_sb[:, k, :], in_=kern_d[128 * k : 128 * k + 128, :])
                nc.gpsimd.dma_start(out=r_sb[:, k, :], in_=r_d[128 * k : 128 * k + 128, :])
            bias_sb = cpool.tile([1, G4], dt.bfloat16, tag="bias")
            nc.gpsimd.dma_start(out=bias_sb, in_=bias_d)
            ones_sb = cpool.tile([1, 128], dt.bfloat16, tag="ones")
            nc.vector.memset(ones_sb, 1.0)
            ident_bf = cpool.tile([128, 128], dt.bfloat16, tag="identb")
            make_identity(nc, ident_bf)
            ident_f = cpool.tile([128, 128], dt.float32, tag="identf")
            make_identity(nc, ident_f)

            # persistent recurrence state (manually double-buffered), per group
            hT_g = []
            c_g = []
            for g in range(NG):
                hT_sb = cpool.tile([128, 2, 128], dt.bfloat16, tag=f"hT{g}")
                nc.vector.memset(hT_sb, 0.0)
                hT_g.append(hT_sb)
                c_sb = cpool.tile([128, 2, 128], dt.float32, tag=f"c{g}")
                nc.vector.memset(c_sb, 0.0)
                c_g.append(c_sb)
            # xz staging: double-buffered TC-step chunks; memset once so the
            # unused partitions (24 of every 32) hold zeros forever.
            xz_chunk = cpool.tile([128, 2, TC, 512], dt.bfloat16, tag="xzs")
            for sl in range(2):
                nc.vector.memset(xz_chunk[:, sl], 0.0)

            with tc.tile_pool(name="p1ps", bufs=1, space="PSUM") as p1ps, \
                 tc.tile_pool(name="p1zps", bufs=1, space="PSUM") as p1zps, \
                 tc.tile_pool(name="p1sb", bufs=3) as p1sb, \
                 tc.tile_pool(name="p2ps", bufs=2, space="PSUM") as p2ps, \
                 tc.tile_pool(name="p2t", bufs=2, space="PSUM") as p2t, \
                 tc.tile_pool(name="p2sb", bufs=3) as p2sb:

                out_dmas_by_tb = {}

                def p1_slices():
                    """Each yield issues one hb-chunk of one (sample, t-block):
                    5 MMs + copy + out-DMA; first chunk also DMA-in + 4
                    transposes building the bf16 lhsT."""
                    state = {}
                    for tb in range(T // 128):
                        for b_i in range(BL):
                            for j in range(HB):
                                if j == 0:
                                    x_sb = p1sb.tile([128, 512], dt.float32, tag="xin")
                                    nc.sync.dma_start(
                                        out=x_sb,
                                        in_=tx_d[b_i * T + tb * 128 : b_i * T + tb * 128 + 128, :],
                                    )
                                    lhs = p1sb.tile([128, 4, 128], dt.bfloat16, tag="lhs")
                                    for k in range(4):
                                        xT = p1ps.tile([128, 128], dt.float32, tag="xT")
                                        nc.tensor.transpose(
                                            xT, x_sb[:, 128 * k : 128 * k + 128], ident_f
                                        )
                                        nc.vector.tensor_copy(lhs[:, k, :], xT)
                                    state["lhs"] = lhs
                                lhs = state["lhs"]
                                ps = p1zps.tile([128, 512], dt.float32, tag="p1z")
                                for k in range(4):
                                    nc.tensor.matmul(
                                        ps,
                                        lhsT=lhs[:, k, :],
                                        rhs=kern_sb[:, k, j * 512 : j * 512 + 512],
                                        start=(k == 0),
                                        stop=False,
                                        skip_group_check=True,
                                    )
                                nc.tensor.matmul(
                                    ps,
                                    lhsT=ones_sb,
                                    rhs=bias_sb[:, j * 512 : j * 512 + 512],
                                    start=False,
                                    stop=True,
                                    skip_group_check=True,
                                )
                                xzo = p1sb.tile([128, 512], dt.bfloat16, tag="xzo")
                                nc.vector.tensor_copy(xzo, ps)
                                od = nc.sync.dma_start(
                                    out=xz_d[tb * 128 : tb * 128 + 128, j, b_i, :], in_=xzo
                                )
                                out_dmas_by_tb.setdefault(tb, []).append(od)
                                yield

                p1_iter = p1_slices()

                def drip(n):
                    for _ in range(n):
                        if next(p1_iter, "done") == "done":
                            return

                # prime: everything for tb 0,1 (covers xz chunks 0..3)
                drip(2 * BL * HB)

                def load_chunk(ci):
                    tb_src = (ci * TC) // 128
                    for hb in range(HB):
                        cd = nc.sync.dma_start(
                            out=xz_chunk[32 * hb : 32 * hb + BL, ci % 2],
                            in_=xz_d[ci * TC : (ci + 1) * TC, hb].rearrange(
                                "t b f -> b t f"
                            ),
                        )
                        for od in out_dmas_by_tb.get(tb_src, []):
                            add_dep_helper(cd.ins, od.ins, sync=True,
                                           reason="xz RAW p1->p2")

                load_chunk(0)

                def emit_injects(t):
                    """open step t's two psum half-banks with xz injects;
                    emitted right after step t-1's RMMs so the PE runs them
                    during the serial-tail stall instead of idling behind
                    the S4T/tcT transposes in the FIFO."""
                    xz_nb = xz_chunk[:, (t // TC) % 2, t % TC, :]
                    hs = []
                    for hv in range(2):
                        zh = p2ps.tile([128, 256], dt.float32,
                                       tag=f"z0h{hv}", name=f"z0h{hv}_{t}")
                        hs.append(zh)
                        nc.tensor.matmul(
                            zh, lhsT=ident_bf,
                            rhs=xz_nb[:, 256 * hv : 256 * hv + 256],
                            start=True, stop=False, skip_group_check=True,
                        )
                    return hs

                pend = emit_injects(0)
                for t in range(T):
                    cur, nxt = t % 2, (t + 1) % 2
                    tc_i, tl = t // TC, t % TC
                    slot = tc_i % 2
                    if tl == 0 and tc_i + 1 < T // TC:
                        load_chunk(tc_i + 1)   # prefetch one chunk ahead
                    xz_sb = xz_chunk[:, slot, tl, :]

                    # phase-major emission: the two groups' chains interleave
                    # inside each engine's strict FIFO instead of serializing.
                    zs, Ss, S4Ts, S4T_sbs, tcts = {}, {}, {}, {}, {}
                    for g in range(NG):
                        hT_sb = hT_g[g]
                        # -- z = xz + h @ R: inject + 16 RMMs into ONE psum bank
                        # (rows outside this group's samples carry bounded junk)
                        if SPLITZ:
                            # injects were pre-emitted last step (pend);
                            # half-B k-loop descending reuses half-A's
                            # last-loaded k3 lhsT slice.
                            halves = pend
                            for hv in range(2):
                                korder = range(4) if hv == 0 else range(3, -1, -1)
                                for k in korder:
                                    for hb in range(HB):
                                        nc.tensor.matmul(
                                            halves[hv][32 * hb : 32 * hb + BL, :],
                                            lhsT=hT_sb[:, cur, 32 * k : 32 * k + BL],
                                            rhs=r_sb[:, k, hb * 512 + 256 * hv : hb * 512 + 256 * hv + 256],
                                            start=False,
                                            stop=(hb == 3 and (k == 3 if hv == 0 else k == 0)),
                                            skip_group_check=True,
                                            tile_position=(0, 32 * hb),
                                        )
                            zs[g] = halves
                            if t + 1 < T:
                                pend = emit_injects(t + 1)
                        else:
                            z = p2ps.tile([128, 512], dt.float32, tag=f"z{g}", name=f"z{g}")
                            zs[g] = z
                            nc.tensor.matmul(
                                z, lhsT=ident_bf, rhs=xz_sb,
                                start=True, stop=False, skip_group_check=True,
                            )
                            # k-outer / strip-inner: consecutive MMs hit different
                            # col strips AND different PSUM rows, so weight loads
                            # overlap in-flight MMs and accumulate RMWs pipeline.
                            for k in range(4):
                                for hb in range(HB):
                                    nc.tensor.matmul(
                                        z[32 * hb : 32 * hb + BL, :],
                                        lhsT=hT_sb[:, cur, 32 * k : 32 * k + BL],
                                        rhs=r_sb[:, k, hb * 512 : hb * 512 + 512],
                                        start=False,
                                        stop=(hb == 3 and k == 3),
                                        skip_group_check=True,
                                        tile_position=(0, 32 * hb),
                                    )

                    # -- PE filler: drip phase-1 slices into the tail stall
                    # (emitted here so drip MMs precede next step's RMMs in
                    # the PE FIFO but run during this step's tail idle) --
                    if t % 4 == 0:
                        drip(1)
                    if ABLATION == "rmm_only":
                        continue
                    for g in range(NG):
                        # -- sigmoid over all gates (one instr, or per half) --
                        S = p2sb.tile([128, 512], dt.float32 if S_FP32 else dt.bfloat16, tag=f"S{g}", name=f"S{g}")
                        Ss[g] = S
                        if SPLITZ:
                            nc.scalar.activation(S[:, 0:256], zs[g][0], AFT.Sigmoid)
                            nc.scalar.activation(S[:, 256:512], zs[g][1], AFT.Sigmoid)
                        else:
                            nc.scalar.activation(S, zs[g], AFT.Sigmoid)
                    for g in range(NG):
                        # -- cell update (DVE) --
                        S, c_sb = Ss[g], c_g[g]
                        w1 = p2sb.tile([128, 128], dt.bfloat16, tag=f"w1{g}", name=f"w1{g}")
                        nc.vector.tensor_scalar(w1, S[:, 0:128], 2.0, 1.0, MULT, SUB)
                        m2 = p2sb.tile([128, 128], dt.float32, tag=f"m2{g}", name=f"m2{g}")
                        nc.vector.tensor_mul(m2, S[:, 128:256], c_sb[:, cur, :])
                        m1 = p2sb.tile([128, 128], dt.float32, tag=f"m1{g}", name=f"m1{g}")
                        nc.vector.tensor_mul(m1, w1, S[:, 256:384])
                        nc.vector.tensor_add(c_sb[:, nxt, :], m1, m2)

                    if ABLATION == "no_hT":
                        continue
                    for g in range(NG):
                        # -- S4^T via PE, copied to SBUF on ScalarE --
                        S = Ss[g]
                        S4T = p2t.tile([128, 128], dt.float32 if S_FP32 else dt.bfloat16, tag="tp", name=f"S4T{g}")
                        S4Ts[g] = S4T
                        nc.tensor.transpose(S4T, S[:, 384:512], ident_f if S_FP32 else ident_bf)
                    for g in range(NG):
                        S4T_sb = p2sb.tile([128, 128], dt.bfloat16, tag=f"s4t{g}", name=f"s4t{g}")
                        S4T_sbs[g] = S4T_sb
                        nc.scalar.copy(S4T_sb, S4Ts[g])
                    for g in range(NG):
                        tc_t = p2sb.tile([128, 128], dt.bfloat16, tag=f"tct{g}", name=f"tct{g}")
                        tcts[g] = tc_t
                        nc.scalar.activation(tc_t, c_g[g][:, nxt, :], AFT.Tanh)
                    for g in range(NG):
                        tcT = p2t.tile([128, 128], dt.bfloat16, tag="tp", name=f"tcT{g}")
                        nc.tensor.transpose(tcT, tcts[g], ident_bf)
                        nc.vector.tensor_mul(hT_g[g][:, nxt, :], S4T_sbs[g], tcT)

                drip(10**9)

            tc.strict_bb_all_engine_barrier()
            hT_f = cpool.tile([128, NG, 128], dt.float32, tag="hTf")
            for g in range(NG):
                nc.vector.tensor_copy(hT_f[:, g, :], hT_g[g][:, T % 2, :])
            for g in range(NG):
                nc.sync.dma_start(out=hT_out_d[g], in_=hT_f[:, g, :])

    nc.compile()
    return nc


_NC_CACHE = None
LAST_RESULTS = None  # BassKernelResults from the most recent run (for profiling)


def _make_in_maps(tx, kern, R, bias):
    kern_perm = np.ascontiguousarray(kern[:, _PERM] * _COLSCALE).astype(BF16)
    r_perm = np.ascontiguousarray(R[:, _PERM] * _COLSCALE).astype(BF16)
    bias_perm = np.ascontiguousarray(bias[_PERM] * _COLSCALE)[None, :].astype(BF16)
    in_maps = []
    for ci in range(NCORES):
        in_maps.append({
            "tx": np.ascontiguousarray(tx[ci * BL : (ci + 1) * BL].reshape(BL * T, D)),
            "kern_perm": kern_perm,
            "r_perm": r_perm,
            "bias_perm": bias_perm,
        })
    return in_maps


def _gather_h(per_core_results):
    h_last = np.empty((B, U), dtype=np.float32)
    GB = BL // NG
    for ci in range(NCORES):
        hT = per_core_results[ci]["hT_out"]          # [NG, jl, 32*hb+b] f32
        for b in range(BL):
            g = b // GB
            h_last[ci * BL + b] = hT[g].reshape(128, HB, 32)[:, :, b].T.reshape(U)
    return h_last


def kernel(tx, kernel, recurrent_kernel, bias, fc_w, fc_b):
    global _NC_CACHE, LAST_RESULTS
    from concourse.bass_utils import run_bass_kernel_spmd

    tx = np.asarray(tx, dtype=np.float32)
    kern = np.asarray(kernel, dtype=np.float32)
    R = np.asarray(recurrent_kernel, dtype=np.float32)
    bias = np.asarray(bias, dtype=np.float32)
    fc_w = np.asarray(fc_w, dtype=np.float32)
    fc_b = np.asarray(fc_b, dtype=np.float32)

    if _NC_CACHE is None:
        _NC_CACHE = _build_bass()
    nc = _NC_CACHE

    in_maps = _make_in_maps(tx, kern, R, bias)
    res = run_bass_kernel_spmd(nc, in_maps, core_ids=list(range(NCORES)))
    LAST_RESULTS = res
    h_last = _gather_h(res.results)

    logits = h_last @ fc_w + fc_b
    e = np.exp(logits - logits.max(axis=1, keepdims=True))
    return (e / e.sum(axis=1, keepdims=True)).astype(np.float32)

